# revision 2
# baseline (speedup 1.0000x reference)
"""Trainium2 Bass kernel for nn_ReasoningLayer (per-token MLP, passthrough pos 0).

  out[:, 0]  = hidden_states[:, 0]
  out[:, i]  = GELU(hidden_states[:, i] @ W1 + b1) @ W2 + b2   for i >= 1

Sharding: pure data parallel over batch — core b computes batch b (2048
tokens), weights replicated. Device works in transposed layout (x^T [D, T])
so both matmuls contract over SBUF partitions with no on-device transposes.

Compute modes:
  "3term" (experimental, NOT default): fp8e4m3 DoubleRow matmuls. Each operand is split
    hi/lo into two e4m3 tensors sharing one power-of-2 scale, and each
    matmul computes W_hi@X_hi + W_lo@X_hi + W_hi@X_lo (the lo@lo term is
    ~2^-8 relative and dropped). DoubleRow contracts K=256 per instruction
    (pair = two adjacent k-tiles) at ~0.5 cyc/out-row — ~2x the bf16 FLOP
    rate — so the 1.5x term overhead still nets out faster than bf16.
    Weight tensors are pre-scaled by 128 on host (keeps U(+-0.044) weights
    out of e4m3's subnormal range); the 1/128 descale is folded into the
    ScalarE activation (MM1) and the DVE bias-add (MM2). The hidden
    activation h is split on device: ScalarE computes GELU into an fp32
    tile, DVE casts it to h_hi (e4m3) and h_lo = h - h_hi.
    End-to-end rel err ~2e-3.
  "bf16" (default): plain bf16 matmuls, 1 cyc/out-row at ~2.8GHz. The
    4-slab body measures 187.0us -- ~100% of the 524288-PE-row roofline,
    i.e. the tensor engine never stalls. rel err ~3.5e-3.
    The 3term fp8 path is numerically validated (rel err ~2.6e-3) and
    DoubleRow MMs in the coarse 4-MM-group structure do hit ~0.5
    cyc/out-row (2x bf16 FLOP rate), but any structure with more MMs per
    PSUM accumulation group collapses to >=2x the per-MM cost on this
    hardware (measured: 2 passes ~190us, 3 passes ~310-330us, invariant
    to operand-AP marching/duplication), so the 1.5x-term scheme does not
    beat bf16 end to end and bf16 is shipped.

MM instructions are ordered pass-major ("marching"): within a PSUM
accumulation group the stationary/moving APs advance monotonically through
SBUF. Back-to-back DoubleRow matmuls that reuse the same operand APs run
~3x slower on this hardware, so term passes iterate over k outer-to-inner.
"""

import numpy as np
import ml_dtypes

B, S, D, H = 8, 2048, 1024, 2048
P = 128
NCORES = 8
TCORE = (B * S) // NCORES  # 2048 tokens per core
TSLAB = 512
NSLAB = TCORE // TSLAB     # 4
DO = D // P                # 8
JO = H // P                # 16
OO = D // P                # 8
SW = 128.0                 # weight pre-scale for e4m3

COMPUTE = "bf16"

E4 = ml_dtypes.float8_e4m3
BF16 = ml_dtypes.bfloat16

_nc_cache = {}


def _build_3term(loopn=None):
    import concourse.bass as bass
    import concourse.mybir as mybir
    import concourse.tile as tile
    from concourse import bacc

    f32, f8 = mybir.dt.float32, mybir.dt.float8e4
    ts = bass.ts
    Gelu = mybir.ActivationFunctionType.Gelu
    DR = mybir.MatmulPerfMode.DoubleRow
    Mult, Add = mybir.AluOpType.mult, mybir.AluOpType.add

    nc = bacc.Bacc("TRN2", target_bir_lowering=False, debug=False,
                   num_devices=NCORES)
    xh = nc.dram_tensor("xh", [D, TCORE], f8, kind="ExternalInput")
    xl = nc.dram_tensor("xl", [D, TCORE], f8, kind="ExternalInput")
    w1h = nc.dram_tensor("w1h", [D, H], f8, kind="ExternalInput")
    w1l = nc.dram_tensor("w1l", [D, H], f8, kind="ExternalInput")
    b1 = nc.dram_tensor("b1", [H], f32, kind="ExternalInput")
    w2h = nc.dram_tensor("w2h", [H, D], f8, kind="ExternalInput")
    w2l = nc.dram_tensor("w2l", [H, D], f8, kind="ExternalInput")
    b2 = nc.dram_tensor("b2", [D], f32, kind="ExternalInput")
    yT = nc.dram_tensor("yT", [D, TCORE], mybir.dt.bfloat16,
                        kind="ExternalOutput")

    with tile.TileContext(nc) as tc:
        with (
            tc.tile_pool(name="w", bufs=1) as wpool,
            tc.tile_pool(name="bias", bufs=1) as bpool,
            tc.tile_pool(name="x", bufs=2) as xpool,
            tc.tile_pool(name="h", bufs=2) as hpool,
            tc.tile_pool(name="hf", bufs=4) as hfpool,
            tc.tile_pool(name="y", bufs=2) as ypool,
            tc.tile_pool(name="ps1", bufs=4, space=bass.MemorySpace.PSUM) as pp1,
            tc.tile_pool(name="ps2", bufs=4, space=bass.MemorySpace.PSUM) as pp2,
        ):
            def load_w(t, nt, name):
                sb = wpool.tile([P, nt, t.shape[1]], f8, name=name)
                tr = t.rearrange("(a b) c -> b a c", b=P)
                for i in range(nt):
                    nc.sync.dma_start(sb[:, i], tr[:, i])
                return sb

            w1h_a = load_w(w1h, DO, "w1h_a")
            w1l_sb = load_w(w1l, DO, "w1l_sb")
            w2h_a = load_w(w2h, JO, "w2h_a")
            w2l_sb = load_w(w2l, JO, "w2l_sb")
            # Duplicate hi-part weights: back-to-back DoubleRow matmuls that
            # re-read the same SBUF APs within a PSUM group run ~3x slower,
            # so each term pass streams from its own copy.
            w1h_b = load_w(w1h, DO, "w1h_b")
            w2h_b = load_w(w2h, JO, "w2h_b")
            b1_sb = bpool.tile([P, JO], f32, name="b1_sb")
            nc.sync.dma_start(b1_sb[:], b1.rearrange("(jo ji) -> ji jo", ji=P))
            b2_sb = bpool.tile([P, OO], f32, name="b2_sb")
            nc.sync.dma_start(b2_sb[:], b2.rearrange("(oo oi) -> oi oo", oi=P))

            xhr = xh.rearrange("(do di) t -> di do t", di=P)
            xlr = xl.rearrange("(do di) t -> di do t", di=P)
            yTr = yT.rearrange("(oo oi) t -> oi oo t", oi=P)

            def body():
                for it in range(NSLAB):
                    xh_a = xpool.tile([P, DO, TSLAB], f8, tag="xh_a")
                    xh_b = xpool.tile([P, DO, TSLAB], f8, tag="xh_b")
                    xl_sb = xpool.tile([P, DO, TSLAB], f8, tag="xl_sb")
                    for do in range(DO):
                        nc.sync.dma_start(xh_a[:, do], xhr[:, do, ts(it, TSLAB)])
                        nc.sync.dma_start(xh_b[:, do], xhr[:, do, ts(it, TSLAB)])
                        nc.sync.dma_start(xl_sb[:, do], xlr[:, do, ts(it, TSLAB)])

                    hh_a = hpool.tile([P, JO, TSLAB], f8, tag="hh_a")
                    hh_b = hpool.tile([P, JO, TSLAB], f8, tag="hh_b")
                    hl_sb = hpool.tile([P, JO, TSLAB], f8, tag="hl_sb")
                    for jt in range(JO):
                        ps = pp1.tile([P, TSLAB], f32, tag="ps1")
                        npair = DO // 2
                        passes = [(w1h_a, xh_a), (w1l_sb, xh_b),
                                  (w1h_b, xl_sb)]
                        for q, (wsb, msb) in enumerate(passes):
                            for kp in range(npair):
                                sl = slice(2 * kp, 2 * kp + 2)
                                nc.tensor.matmul(
                                    ps[:], wsb[:, sl, ts(jt, P)], msb[:, sl],
                                    start=(q == 0 and kp == 0),
                                    stop=(q == 2 and kp == npair - 1),
                                    perf_mode=DR)
                        hf = hfpool.tile([P, TSLAB], f32, tag="hf")
                        nc.scalar.activation(hf[:], ps[:], Gelu,
                                             bias=b1_sb[:, ts(jt, 1)],
                                             scale=1.0 / SW)
                        nc.vector.tensor_copy(hh_a[:, jt], hf[:])
                        nc.vector.tensor_copy(hh_b[:, jt], hf[:])
                        nc.vector.tensor_sub(hl_sb[:, jt], hf[:], hh_a[:, jt])

                    y_sb = ypool.tile([P, OO, TSLAB], mybir.dt.bfloat16,
                                      tag="y_sb")
                    for ot in range(OO):
                        ps2 = pp2.tile([P, TSLAB], f32, tag="ps2")
                        npair2 = JO // 2
                        passes2 = [(w2h_a, hh_a), (w2l_sb, hh_b),
                                   (w2h_b, hl_sb)]
                        for q, (wsb, msb) in enumerate(passes2):
                            for jp in range(npair2):
                                sl = slice(2 * jp, 2 * jp + 2)
                                nc.tensor.matmul(
                                    ps2[:], wsb[:, sl, ts(ot, P)], msb[:, sl],
                                    start=(q == 0 and jp == 0),
                                    stop=(q == 2 and jp == npair2 - 1),
                                    perf_mode=DR)
                        nc.vector.tensor_scalar(y_sb[:, ot], ps2[:], 1.0 / SW,
                                                b2_sb[:, ts(ot, 1)], Mult, Add)
                    for oo in range(OO):
                        nc.sync.dma_start(yTr[:, oo, ts(it, TSLAB)], y_sb[:, oo])

            if loopn is None:
                body()
            else:
                with tc.For_i(0, loopn) as _i:
                    body()

    nc.compile()
    return nc


def _build_bf16(loopn=None):
    import concourse.bass as bass
    import concourse.mybir as mybir
    import concourse.tile as tile
    from concourse import bacc

    f32, bf = mybir.dt.float32, mybir.dt.bfloat16
    ts = bass.ts
    Gelu = mybir.ActivationFunctionType.Gelu

    nc = bacc.Bacc("TRN2", target_bir_lowering=False, debug=False,
                   num_devices=NCORES)
    xT = nc.dram_tensor("xT", [D, TCORE], bf, kind="ExternalInput")
    w1 = nc.dram_tensor("w1", [D, H], bf, kind="ExternalInput")
    b1 = nc.dram_tensor("b1", [H], f32, kind="ExternalInput")
    w2 = nc.dram_tensor("w2", [H, D], bf, kind="ExternalInput")
    b2 = nc.dram_tensor("b2", [D], f32, kind="ExternalInput")
    yT = nc.dram_tensor("yT", [D, TCORE], f32, kind="ExternalOutput")

    with tile.TileContext(nc) as tc:
        with (
            tc.tile_pool(name="w", bufs=1) as wpool,
            tc.tile_pool(name="bias", bufs=1) as bpool,
            tc.tile_pool(name="x", bufs=2) as xpool,
            tc.tile_pool(name="h", bufs=2) as hpool,
            tc.tile_pool(name="y", bufs=2) as ypool,
            tc.tile_pool(name="ps1", bufs=4, space=bass.MemorySpace.PSUM) as pp1,
            tc.tile_pool(name="ps2", bufs=4, space=bass.MemorySpace.PSUM) as pp2,
        ):
            w1_sb = wpool.tile([P, DO, H], bf, name="w1_sb")
            w1r = w1.rearrange("(do di) j -> di do j", di=P)
            for do in range(DO):
                nc.sync.dma_start(w1_sb[:, do], w1r[:, do])
            w2_sb = wpool.tile([P, JO, D], bf, name="w2_sb")
            w2r = w2.rearrange("(jo ji) o -> ji jo o", ji=P)
            for jo in range(JO):
                nc.sync.dma_start(w2_sb[:, jo], w2r[:, jo])
            b1_sb = bpool.tile([P, JO], f32, name="b1_sb")
            nc.sync.dma_start(b1_sb[:], b1.rearrange("(jo ji) -> ji jo", ji=P))
            b2_sb = bpool.tile([P, OO], f32, name="b2_sb")
            nc.sync.dma_start(b2_sb[:], b2.rearrange("(oo oi) -> oi oo", oi=P))

            xTr = xT.rearrange("(do di) t -> di do t", di=P)
            yTr = yT.rearrange("(oo oi) t -> oi oo t", oi=P)

            def body():
                for it in range(NSLAB):
                    x_sb = xpool.tile([P, DO, TSLAB], bf, tag="x_sb")
                    for do in range(DO):
                        nc.sync.dma_start(x_sb[:, do], xTr[:, do, ts(it, TSLAB)])
                    h_sb = hpool.tile([P, JO, TSLAB], bf, tag="h_sb")
                    for jt in range(JO):
                        ps = pp1.tile([P, TSLAB], f32, tag="ps1")
                        for kt in range(DO):
                            nc.tensor.matmul(ps[:], w1_sb[:, kt, ts(jt, P)],
                                             x_sb[:, kt], start=(kt == 0),
                                             stop=(kt == DO - 1))
                        nc.scalar.activation(h_sb[:, jt], ps[:], Gelu,
                                             bias=b1_sb[:, ts(jt, 1)])
                    y_sb = ypool.tile([P, OO, TSLAB], f32, tag="y_sb")
                    for ot in range(OO):
                        ps2 = pp2.tile([P, TSLAB], f32, tag="ps2")
                        for jt in range(JO):
                            nc.tensor.matmul(ps2[:], w2_sb[:, jt, ts(ot, P)],
                                             h_sb[:, jt], start=(jt == 0),
                                             stop=(jt == JO - 1))
                        nc.vector.tensor_scalar_add(y_sb[:, ot], ps2[:],
                                                    b2_sb[:, ts(ot, 1)])
                    for oo in range(OO):
                        nc.sync.dma_start(yTr[:, oo, ts(it, TSLAB)], y_sb[:, oo])

            if loopn is None:
                body()
            else:
                with tc.For_i(0, loopn) as _i:
                    body()

    nc.compile()
    return nc


def _get_nc(compute=COMPUTE, loopn=None):
    key = (compute, loopn)
    if key not in _nc_cache:
        _nc_cache[key] = (_build_3term(loopn) if compute == "3term"
                          else _build_bf16(loopn))
    return _nc_cache[key]


def _in_maps(hidden_states, W1, b1, W2, b2, compute=COMPUTE):
    hidden_states = np.asarray(hidden_states, np.float32)
    b1c = np.ascontiguousarray(np.asarray(b1, np.float32))
    b2c = np.ascontiguousarray(np.asarray(b2, np.float32))
    if compute == "3term":
        W1s = np.asarray(W1, np.float32) * SW
        W2s = np.asarray(W2, np.float32) * SW
        w1h = W1s.astype(E4)
        w1l = np.ascontiguousarray((W1s - w1h.astype(np.float32)).astype(E4))
        w1h = np.ascontiguousarray(w1h)
        w2h = W2s.astype(E4)
        w2l = np.ascontiguousarray((W2s - w2h.astype(np.float32)).astype(E4))
        w2h = np.ascontiguousarray(w2h)
        maps = []
        for c in range(NCORES):
            # order='C' so the transposed view is materialized contiguously
            xT = hidden_states[c].T.astype(np.float32, order="C")  # [D, T]
            xh = xT.astype(E4)
            xl = np.ascontiguousarray((xT - xh.astype(np.float32)).astype(E4))
            maps.append({"xh": np.ascontiguousarray(xh), "xl": xl,
                         "w1h": w1h, "w1l": w1l, "w2h": w2h, "w2l": w2l,
                         "b1": b1c, "b2": b2c})
        return maps
    W1c = np.ascontiguousarray(np.asarray(W1).astype(BF16))
    W2c = np.ascontiguousarray(np.asarray(W2).astype(BF16))
    maps = []
    for c in range(NCORES):
        xT_c = hidden_states[c].T.astype(BF16, order="C")
        maps.append({"xT": xT_c, "w1": W1c, "b1": b1c, "w2": W2c, "b2": b2c})
    return maps


def _run(hidden_states, W1, b1, W2, b2, compute=COMPUTE):
    from concourse import bass_utils

    nc = _get_nc(compute)
    maps = _in_maps(hidden_states, W1, b1, W2, b2, compute)
    res = bass_utils.run_bass_kernel_spmd(
        nc, maps, core_ids=list(range(NCORES))
    )
    hidden_states = np.asarray(hidden_states, np.float32)
    out = np.empty((B, S, D), np.float32)
    for c in range(NCORES):
        out[c] = res.results[c]["yT"].T.astype(np.float32)
    out[:, 0, :] = hidden_states[:, 0, :]
    return out, res


def kernel(hidden_states, W1, b1, W2, b2):
    out, _ = _run(hidden_states, W1, b1, W2, b2)
    return out
